# revision 1
# baseline (speedup 1.0000x reference)
"""Trainium2 Bass kernel for nn_ExampleModel_1116691497724 (moe_routing).

Math: the reference returns log_softmax_T( sum_D(moe_out) ), and sum_D
collapses the expert FFN to a dot product:
    sum_d (h @ W2[e] + b2[e]) = h . w2sum[e] + sum(b2[e]),  w2sum[e] = W2[e] @ 1
    (x @ W1[e] + b1[e]) . w2sum[e] = x . v[e] + c[e]
with v[e] = W1[e] @ w2sum[e]  (a [D] vector) and scalar
c[e] = b1[e].w2sum[e] + sum(b2[e]).  Then per token:
    s_e = x . v[e] + c[e],  logits = x @ Wg
    moe_sum = max(softmax(logits)) * s_argmax(logits)
    out = log_softmax over tokens (per batch row) of moe_sum.

Distribution over 8 cores, two launches (measured: a single ncfw collective
costs ~65us of barrier/trigger latency on this runtime — far more than a
second launch's fixed ~17us, so the 16KB cross-core combine happens on the
host between launches; the host does only that partial sum, all real math
stays on device):
  launch A (expert-parallel over H): core c reduces W2[:, 128c:128c+128, :]
    and computes partial v from the matching W1 columns (f32r stream after a
    rounding pass) -> outputs [v0 | v1 | c0 c1] partials (16KB); host sums.
  launch B (token-parallel): core c owns batch row c%4 (512 tokens): logits
    stream in fp32 (exact — argmax ties must match the reference), s stream
    in f32r, gate/select per token after a PE transpose, row log_softmax via
    PE transposes (no cross-partition DMA).  Host takes rows from cores 0..3.

Scheduling: stationary matmul operands are tiny (M<=4) so LDWEIGHTS is
negligible; fp32 streams at 4 cycles/row, f32r at 1.  Big loads alternate the
two HWDGE rings (SP via nc.sync, ACT via nc.scalar) for concurrency.  The d
axis is decomposed as d = p*16 + n so the flat v vector loads into [128,16]
tiles with contiguous per-partition runs.
"""

import sys

import numpy as np

for _p in ("/opt/trn_rl_repo",):
    if _p not in sys.path:
        sys.path.append(_p)

import concourse.bass as bass  # noqa: E402
import concourse.mybir as mybir  # noqa: E402
import concourse.tile as tile  # noqa: E402
from concourse import bacc, bass_utils  # noqa: E402
from concourse.masks import make_identity  # noqa: E402

# Problem shape (hardcoded per spec).
B, T, D, H, E = 4, 512, 2048, 1024, 2
P = 128
NCORES = 8
TB = T  # tokens per core = one batch row
NB = D // P  # 16 d-blocks
HC = H // NCORES  # 128 h-chunk per expert per core
NG = TB // P  # 4 token groups per core
DC = D // NCORES  # 256 b2 columns per core
VK = 4  # v computed in VK chunks of D/VK columns
F32 = mybir.dt.float32
F32R = mybir.dt.float32r
AX = mybir.AxisListType
AF = mybir.ActivationFunctionType
ALU = mybir.AluOpType

VPART = 2 * D + 2  # launch A output: v0 | v1 | c0 c1
BF16 = mybir.dt.bfloat16
BF16_W = False  # bf16 W1/W2 saves only ~2us but costs 13x accuracy; keep f32


def emit_phase_a(nc, tc, io):
    """w2sum + partial v for this core's H-chunk -> vpart [1, 2D+2]."""
    w1t, w2r, b1c, b2c, vout = io["w1t"], io["w2r"], io["b1c"], io["b2c"], io["vout"]
    with (
        tc.tile_pool(name="main", bufs=1) as pool,
        tc.tile_pool(name="psum", bufs=1, space="PSUM") as psum,
    ):
        # DMA plan: tiny contiguous bias rows FIRST on the sync ring (so no
        # DVE op ever head-of-line blocks on them), then W2 halves (they gate
        # the reduce), then W1 split over all three queues.  W1 goes straight
        # into an f32r tile (w1t is declared float32r) — no cast pass.
        HD = D // 2
        WDT = BF16 if BF16_W else F32
        VDT = BF16 if BF16_W else F32R
        b1_sb = pool.tile([1, E * HC], F32)
        nc.sync.dma_start(b1_sb[:], b1c)
        b2_sb = pool.tile([1, E * DC], F32)
        nc.sync.dma_start(b2_sb[:], b2c)
        w2_sb = pool.tile([P, E, D], WDT)
        w1r = pool.tile([P, E, D], VDT)
        for h in range(2):
            nc.sync.dma_start(w2_sb[:, 0, h * HD : (h + 1) * HD], w2r[0, :, h * HD : (h + 1) * HD])
            nc.scalar.dma_start(w2_sb[:, 1, h * HD : (h + 1) * HD], w2r[1, :, h * HD : (h + 1) * HD])
        for h in range(2):
            nc.sync.dma_start(w1r[:, 0, h * HD : (h + 1) * HD], w1t[0, :, h * HD : (h + 1) * HD])
            nc.scalar.dma_start(w1r[:, 1, h * HD : (h + 1) * HD], w1t[1, :, h * HD : (h + 1) * HD])

        # b1 row -> partition-major [128, E] via PE transpose (identity [1,1])
        one1 = pool.tile([1, 1], F32)
        nc.gpsimd.memset(one1[:], 1.0)
        b1t_ps = psum.tile([P, E], F32)
        for e in range(E):
            nc.tensor.transpose(
                b1t_ps[:, e : e + 1], b1_sb[0:1, e * HC : (e + 1) * HC], one1[:]
            )
        b1p = pool.tile([P, E], F32)
        nc.vector.tensor_copy(b1p[:], b1t_ps[:])

        w2h = pool.tile([P, 2 * E], F32)
        w2s = pool.tile([P, E], F32)
        for e in range(E):
            for h in range(2):
                nc.vector.reduce_sum(
                    w2h[:, 2 * e + h : 2 * e + h + 1],
                    w2_sb[:, e, h * HD : (h + 1) * HD],
                    axis=AX.X,
                )
            nc.vector.tensor_add(
                w2s[:, e : e + 1], w2h[:, 2 * e : 2 * e + 1], w2h[:, 2 * e + 1 : 2 * e + 2]
            )
        w2s_r = pool.tile([P, E], VDT)
        nc.vector.tensor_copy(w2s_r[:], w2s[:])
        b2s = pool.tile([1, E], F32)
        for e in range(E):
            nc.vector.reduce_sum(
                b2s[0:1, e : e + 1], b2_sb[0:1, e * DC : (e + 1) * DC], axis=AX.X
            )

        pay = pool.tile([1, VPART], F32)
        b1dot = psum.tile([1, E], F32)
        DK = D // VK
        for e in range(E):
            for k in range(VK):
                vch = psum.tile([1, DK], F32, name="vch", tag="vch", bufs=2)
                nc.tensor.matmul(
                    vch[:],
                    w2s_r[:, e : e + 1],
                    w1r[:, e, k * DK : (k + 1) * DK],
                    start=True,
                    stop=True,
                )
                dst = pay[0:1, e * D + k * DK : e * D + (k + 1) * DK]
                if k % 2 == 0:
                    nc.vector.tensor_copy(dst, vch[:])
                else:
                    nc.scalar.copy(dst, vch[:])
            nc.tensor.matmul(
                b1dot[0:1, e : e + 1],
                w2s[:, e : e + 1],
                b1p[:, e : e + 1],
                start=True,
                stop=True,
            )
            nc.vector.tensor_add(
                pay[0:1, 2 * D + e : 2 * D + e + 1],
                b1dot[0:1, e : e + 1],
                b2s[0:1, e : e + 1],
            )
        nc.sync.dma_start(vout[:], pay[:])


def emit_phase_b(nc, tc, io):
    """logits (fp32) + s (f32r) streams, gate/select, row log_softmax."""
    xt, wgt, vin, out = io["xt"], io["wgt"], io["vin"], io["out"]
    rings = [nc.sync, nc.scalar]
    with (
        tc.tile_pool(name="main", bufs=1) as pool,
        tc.tile_pool(name="psum", bufs=1, space="PSUM") as psum,
    ):
        # v and Wg arrive as [16, 128] n-major rows (contiguous 512B per
        # partition = few fast packets; a partition-major load would emit 64B
        # packets and clog a queue for several us) and get transposed on the
        # idle PE into the [128, 16] layout the stationary operand needs
        vrow = pool.tile([16, E * P], F32)
        for e in range(E):
            nc.sync.dma_start(
                vrow[:, e * P : (e + 1) * P],
                vin[0:1, e * D : (e + 1) * D].rearrange("x (n p) -> n (x p)", p=P),
            )
        wgr = pool.tile([16, E * P], F32)
        for e in range(E):
            nc.scalar.dma_start(wgr[:, e * P : (e + 1) * P], wgt[e])
        csum = pool.tile([1, E], F32)
        nc.gpsimd.dma_start(csum[:], vin[0:1, 2 * D : 2 * D + E])

        x_sb = pool.tile([P, NB, TB], F32)
        xv = xt.rearrange("(n p) t -> p n t", p=P)  # d = n*128 + p
        qs = [nc.sync, nc.scalar]
        chunks = [
            (0, 0, 1), (1, 1, 2),
            (0, 2, 4), (1, 4, 6),
            (0, 6, 9), (1, 9, 12),
            (0, 12, 14), (1, 14, 16),
        ]
        for q, lo, hi in chunks:
            qs[q].dma_start(x_sb[:, lo:hi, :], xv[:, lo:hi, :])

        # preload ACT tables (Exp, Ln) off the critical path; keep ALL copy
        # work off the scalar engine so these tables are never evicted
        warm = pool.tile([1, 2], F32)
        nc.gpsimd.memset(warm[:], 1.0)
        wz = pool.tile([1, 2], F32)
        nc.scalar.activation(wz[:], warm[:], AF.Exp)
        nc.scalar.activation(wz[:], warm[:], AF.Ln)

        ident = pool.tile([P, P], F32)
        make_identity(nc, ident[:])
        # m4[p, n, :] = [wg0 wg1 v0 v1] for d-block n (d = n*128 + p): one
        # M=4 fp32 stream computes logits AND s together (fp32 matmul cost is
        # per streamed row, independent of stationary columns)
        m4 = pool.tile([P, NB, 4], F32)
        for e in range(E):
            wtp = psum.tile([P, NB], F32, name=f"wtp_{e}", tag="tp16", bufs=2)
            nc.tensor.transpose(wtp[:], wgr[:, e * P : (e + 1) * P], ident[0:16, 0:16])
            nc.vector.tensor_copy(m4[:, :, e : e + 1], wtp[:, :, None])
            vtp = psum.tile([P, NB], F32, name=f"vtp_{e}", tag="tp16", bufs=2)
            nc.tensor.transpose(vtp[:], vrow[:, e * P : (e + 1) * P], ident[0:16, 0:16])
            nc.vector.tensor_copy(m4[:, :, 2 + e : 3 + e], vtp[:, :, None])
        # c broadcast tile: [0, 0, c0, c1] on every partition
        cb4 = pool.tile([P, 4], F32)
        nc.gpsimd.memset(cb4[:, 0:2], 0.0)
        nc.gpsimd.partition_broadcast(cb4[:, 2:4], csum[0:1, :])

        ps4 = psum.tile([4, TB], F32)
        for n in range(NB):
            nc.tensor.matmul(
                ps4[:], m4[:, n, :], x_sb[:, n, :], start=(n == 0), stop=(n == NB - 1)
            )
        sbl = pool.tile([4, TB], F32)
        nc.vector.tensor_copy(sbl[:], ps4[:])

        moe_sb = pool.tile([P, NG], F32)
        for g in range(NG):
            tpl = psum.tile([P, 4], F32, name=f"tpl_{g}", tag="tp", bufs=2)
            nc.tensor.transpose(tpl[:], sbl[0:4, g * P : (g + 1) * P], ident[0:4, 0:4])
            t4 = pool.tile([P, 4], F32, name=f"t4_{g}")
            nc.vector.tensor_add(t4[:], tpl[:], cb4[:])  # adds c to the s cols
            negm = pool.tile([P, 1], F32, name=f"negm_{g}")
            nc.vector.reduce_max(negm[:], t4[:, 0:2], axis=AX.X, negate=True)
            z = pool.tile([P, E], F32, name=f"z_{g}")
            den = pool.tile([P, 1], F32, name=f"den_{g}")
            nc.scalar.activation(z[:], t4[:, 0:2], AF.Exp, bias=negm[:], accum_out=den[:])
            rec = pool.tile([P, 1], F32, name=f"rec_{g}")
            nc.vector.reciprocal(rec[:], den[:])
            zmax = pool.tile([P, 1], F32, name=f"zmax_{g}")
            nc.vector.reduce_max(zmax[:], z[:], axis=AX.X)
            gate = pool.tile([P, 1], F32, name=f"gate_{g}")
            nc.vector.tensor_mul(gate[:], zmax[:], rec[:])
            mask = pool.tile([P, 1], F32, name=f"mask_{g}")
            nc.vector.tensor_tensor(mask[:], t4[:, 0:1], t4[:, 1:2], op=ALU.is_ge)
            sdiff = pool.tile([P, 1], F32, name=f"sdiff_{g}")
            nc.vector.tensor_sub(sdiff[:], t4[:, 2:3], t4[:, 3:4])
            ssel = pool.tile([P, 1], F32, name=f"ssel_{g}")
            nc.vector.tensor_mul(ssel[:], mask[:], sdiff[:])
            nc.vector.tensor_add(ssel[:], ssel[:], t4[:, 3:4])
            nc.vector.tensor_mul(moe_sb[:, g : g + 1], gate[:], ssel[:])

        # row log_softmax over all 512 tokens, via PE transposes
        tp4 = psum.tile([NG, P], F32)
        nc.tensor.transpose(tp4[:], moe_sb[:], ident[:])
        sb4t = pool.tile([NG, P], F32)
        nc.vector.tensor_copy(sb4t[:], tp4[:])
        m4p = pool.tile([NG, 1], F32)
        nc.vector.reduce_max(m4p[:], sb4t[:], axis=AX.X)
        m1p = psum.tile([1, NG], F32, name="m1p", tag="t1", bufs=2)
        nc.tensor.transpose(m1p[:], m4p[:], ident[0:NG, 0:NG])
        negm2 = pool.tile([1, 1], F32)
        nc.vector.reduce_max(negm2[:], m1p[:], axis=AX.X, negate=True)
        negm4 = pool.tile([NG, 1], F32)
        nc.gpsimd.partition_broadcast(negm4[:], negm2[:])
        e4 = pool.tile([NG, P], F32)
        s4 = pool.tile([NG, 1], F32)
        nc.scalar.activation(e4[:], sb4t[:], AF.Exp, bias=negm4[:], accum_out=s4[:])
        # reload the Ln table NOW (the Exp uses above evicted it) so the real
        # Ln below table-hits; overlaps the transpose+reduce on other engines
        nc.scalar.activation(wz[:], warm[:], AF.Ln)
        s1p = psum.tile([1, NG], F32, name="s1p", tag="t1", bufs=2)
        nc.tensor.transpose(s1p[:], s4[:], ident[0:NG, 0:NG])
        ssum = pool.tile([1, 1], F32)
        nc.vector.reduce_sum(ssum[:], s1p[:], axis=AX.X)
        logs = pool.tile([1, 1], F32)
        nc.scalar.activation(logs[:], ssum[:], AF.Ln)
        shift = pool.tile([1, 1], F32)
        nc.vector.tensor_sub(shift[:], negm2[:], logs[:])
        shift4 = pool.tile([NG, 1], F32)
        nc.gpsimd.partition_broadcast(shift4[:], shift[:])
        res4 = pool.tile([NG, P], F32)
        nc.vector.tensor_scalar_add(res4[:], sb4t[:], shift4[:])
        nc.sync.dma_start(out.rearrange("x (g p) -> g (x p)", p=P), res4[:])


_CACHED = {}


def build_program(which):
    if which in _CACHED:
        return _CACHED[which]
    nc = bacc.Bacc(
        "TRN2",
        target_bir_lowering=False,
        debug=False,
        enable_asserts=False,
        num_devices=NCORES,
    )
    if which == "a":
        io = {
            "w1t": nc.dram_tensor(
                "w1t", [E, HC, D], BF16 if BF16_W else F32R, kind="ExternalInput"
            ).ap(),
            "w2r": nc.dram_tensor(
                "w2r", [E, HC, D], BF16 if BF16_W else F32, kind="ExternalInput"
            ).ap(),
            "b1c": nc.dram_tensor("b1c", [1, E * HC], F32, kind="ExternalInput").ap(),
            "b2c": nc.dram_tensor("b2c", [1, E * DC], F32, kind="ExternalInput").ap(),
            "vout": nc.dram_tensor("vout", [1, VPART], F32, kind="ExternalOutput").ap(),
        }
        emit = emit_phase_a
    else:
        io = {
            "xt": nc.dram_tensor("xt", [D, TB], F32, kind="ExternalInput").ap(),
            "wgt": nc.dram_tensor("wgt", [E, NB, P], F32, kind="ExternalInput").ap(),
            "vin": nc.dram_tensor("vin", [1, VPART], F32, kind="ExternalInput").ap(),
            "out": nc.dram_tensor("out", [1, TB], F32, kind="ExternalOutput").ap(),
        }
        emit = emit_phase_b
    with tile.TileContext(nc) as tc:
        emit(nc, tc, io)
    nc.compile()
    _CACHED[which] = nc
    return nc


def shard_inputs_a(Wg, W1, b1, W2, b2):
    if BF16_W:
        import ml_dtypes

        wdt = ml_dtypes.bfloat16
    else:
        wdt = np.float32
    W1 = np.asarray(W1, np.float32)
    b1 = np.asarray(b1, np.float32)
    W2 = np.asarray(W2, np.float32)
    b2 = np.asarray(b2, np.float32)
    in_maps = []
    for c in range(NCORES):
        hs, he = c * HC, (c + 1) * HC
        in_maps.append(
            {
                "w1t": np.ascontiguousarray(W1[:, :, hs:he].transpose(0, 2, 1).astype(wdt)),
                "w2r": np.ascontiguousarray(W2[:, hs:he, :].astype(wdt)),
                "b1c": np.ascontiguousarray(b1[:, hs:he].reshape(1, E * HC)),
                "b2c": np.ascontiguousarray(
                    b2[:, c * DC : (c + 1) * DC].reshape(1, E * DC)
                ),
            }
        )
    return in_maps


def shard_inputs_b(x, Wg, vpart_sum):
    x = np.asarray(x, np.float32).reshape(B * T, D)
    Wg = np.asarray(Wg, np.float32)
    # wgt[p, n*2+e] = Wg[p*16+n, e]  (d = p*16 + n decomposition)
    # wgt[e, n, p] = Wg[n*128+p, e]  (d = n*128 + p decomposition)
    wgt = np.ascontiguousarray(Wg.reshape(NB, P, E).transpose(2, 0, 1))
    in_maps = []
    for c in range(NCORES):
        row = c % B
        in_maps.append(
            {
                "xt": np.ascontiguousarray(x[row * TB : (row + 1) * TB, :].T),
                "wgt": wgt,
                "vin": vpart_sum,
            }
        )
    return in_maps


def run_a(in_maps, **kwargs):
    return bass_utils.run_bass_kernel_spmd(
        build_program("a"), in_maps, core_ids=list(range(NCORES)), **kwargs
    )


def run_b(in_maps, **kwargs):
    return bass_utils.run_bass_kernel_spmd(
        build_program("b"), in_maps, core_ids=list(range(NCORES)), **kwargs
    )


def kernel(x, Wg, W1, b1, W2, b2):
    res_a = run_a(shard_inputs_a(Wg, W1, b1, W2, b2))
    # cross-core combine: sum of the 8 per-core partials (the gather/reshard
    # step between the two launches; 16KB, no model math beyond the reduction)
    vpart = np.sum([res_a.results[c]["vout"] for c in range(NCORES)], axis=0)
    vpart = np.ascontiguousarray(vpart, np.float32)
    res_b = run_b(shard_inputs_b(x, Wg, vpart))
    return np.concatenate([res_b.results[b]["out"] for b in range(B)], axis=0)



# revision 2
# speedup vs baseline: 1.1492x; 1.1492x over previous
"""Trainium2 Bass kernel for nn_ExampleModel_1116691497724 (moe_routing).

Math: the reference returns log_softmax_T( sum_D(moe_out) ), and sum_D
collapses the expert FFN to a dot product:
    sum_d (h @ W2[e] + b2[e]) = h . w2sum[e] + sum(b2[e]),  w2sum[e] = W2[e] @ 1
    (x @ W1[e] + b1[e]) . w2sum[e] = x . v[e] + c[e]
with v[e] = W1[e] @ w2sum[e]  (a [D] vector) and scalar
c[e] = b1[e].w2sum[e] + sum(b2[e]).  Then per token:
    s_e = x . v[e] + c[e],  logits = x @ Wg
    moe_sum = max(softmax(logits)) * s_argmax(logits)
    out = log_softmax over tokens (per batch row) of moe_sum.

Distribution over 8 cores, two launches (a single ncfw collective costs far
more in barrier/trigger latency on this runtime than a second launch, so the
16KB cross-core combine happens on the host between launches; the host does
only that partial sum, all real math stays on device):
  launch A (expert-parallel over H): core c reduces W2[:, 128c:128c+128, :]
    and computes partial v from the matching W1 columns -> outputs
    [v0 | v1 | c0 c1] partials (16KB); host sums.
  launch B (token-parallel): core c owns batch row c%4 (512 tokens); one
    f32r M=4 stream computes logits AND s per token (1 cycle/row vs fp32's
    4); gate/select vectorized across all 4 token groups via
    gate = 1/(1+exp(-|l0-l1|)); row log_softmax via PE transposes.

All big HBM tensors are laid out host-side to exactly match SBUF layout
([128, free] with long contiguous per-partition runs) so each dma_start
emits few large descriptors and streams near HBM line rate.
"""

import sys

import numpy as np

for _p in ("/opt/trn_rl_repo",):
    if _p not in sys.path:
        sys.path.append(_p)

import concourse.bass as bass  # noqa: E402
import concourse.mybir as mybir  # noqa: E402
import concourse.tile as tile  # noqa: E402
from concourse import bacc, bass_utils  # noqa: E402
from concourse.masks import make_identity  # noqa: E402

# Problem shape (hardcoded per spec).
B, T, D, H, E = 4, 512, 2048, 1024, 2
P = 128
NCORES = 8
TB = T  # tokens per core = one batch row
NB = D // P  # 16 d-blocks
HC = H // NCORES  # 128 h-chunk per expert per core
NG = TB // P  # 4 token groups per core
DC = D // NCORES  # 256 b2 columns per core
VK = 4  # v computed in VK chunks of D/VK columns
F32 = mybir.dt.float32
F32R = mybir.dt.float32r
AX = mybir.AxisListType
AF = mybir.ActivationFunctionType
ALU = mybir.AluOpType

VPART = 2 * D + 2  # launch A output: v0 | v1 | c0 c1


def emit_phase_a(nc, tc, io):
    """w2sum + partial v for this core's H-chunk -> vpart [1, 2D+2]."""
    w1c, w2c, b1c, b2c, vout = io["w1c"], io["w2c"], io["b1c"], io["b2c"], io["vout"]
    rings = [nc.sync, nc.scalar]
    with (
        tc.tile_pool(name="main", bufs=1) as pool,
        tc.tile_pool(name="psum", bufs=1, space="PSUM") as psum,
    ):
        # tiny bias rows on the SWDGE ring so they never head-of-line block
        # the big HWDGE streams
        b1_sb = pool.tile([1, E * HC], F32)
        nc.gpsimd.dma_start(b1_sb[:], b1c)
        b2_sb = pool.tile([1, E * DC], F32)
        nc.gpsimd.dma_start(b2_sb[:], b2c)

        # W2 first (gates the reduce -> w2sum -> everything), then W1.
        # Both are host-prearranged [128, E*D] so each chunk is a 4KB/partition
        # contiguous run; chunks alternate the two HWDGE rings.
        CH = E * D // 4  # 1024 cols per chunk
        w2_sb = pool.tile([P, E * D], F32)
        w1r = pool.tile([P, E * D], F32R)
        for k in range(4):
            rings[k % 2].dma_start(w2_sb[:, k * CH : (k + 1) * CH], w2c[:, k * CH : (k + 1) * CH])
        for k in range(4):
            rings[k % 2].dma_start(w1r[:, k * CH : (k + 1) * CH], w1c[:, k * CH : (k + 1) * CH])

        # b1 row -> partition-major [128, E] via PE transpose (identity [1,1])
        one1 = pool.tile([1, 1], F32)
        nc.gpsimd.memset(one1[:], 1.0)
        b1t_ps = psum.tile([P, E], F32)
        for e in range(E):
            nc.tensor.transpose(
                b1t_ps[:, e : e + 1], b1_sb[0:1, e * HC : (e + 1) * HC], one1[:]
            )
        b1p = pool.tile([P, E], F32)
        nc.vector.tensor_copy(b1p[:], b1t_ps[:])

        # per-chunk reduce as each W2 chunk lands (overlaps the W1 DMA)
        w2h = pool.tile([P, 4], F32)
        for k in range(4):
            nc.vector.reduce_sum(
                w2h[:, k : k + 1], w2_sb[:, k * CH : (k + 1) * CH], axis=AX.X
            )
        w2s = pool.tile([P, E], F32)
        for e in range(E):
            nc.vector.tensor_add(
                w2s[:, e : e + 1], w2h[:, 2 * e : 2 * e + 1], w2h[:, 2 * e + 1 : 2 * e + 2]
            )
        w2s_r = pool.tile([P, E], F32R)
        nc.vector.tensor_copy(w2s_r[:], w2s[:])
        b2s = pool.tile([1, E], F32)
        for e in range(E):
            nc.vector.reduce_sum(
                b2s[0:1, e : e + 1], b2_sb[0:1, e * DC : (e + 1) * DC], axis=AX.X
            )

        pay = pool.tile([1, VPART], F32)
        b1dot = psum.tile([1, E], F32)
        DK = D // VK
        for e in range(E):
            for k in range(VK):
                vch = psum.tile([1, DK], F32, name="vch", tag="vch", bufs=2)
                nc.tensor.matmul(
                    vch[:],
                    w2s_r[:, e : e + 1],
                    w1r[:, e * D + k * DK : e * D + (k + 1) * DK],
                    start=True,
                    stop=True,
                )
                dst = pay[0:1, e * D + k * DK : e * D + (k + 1) * DK]
                if k % 2 == 0:
                    nc.vector.tensor_copy(dst, vch[:])
                else:
                    nc.scalar.copy(dst, vch[:])
            nc.tensor.matmul(
                b1dot[0:1, e : e + 1],
                w2s[:, e : e + 1],
                b1p[:, e : e + 1],
                start=True,
                stop=True,
            )
            nc.vector.tensor_add(
                pay[0:1, 2 * D + e : 2 * D + e + 1],
                b1dot[0:1, e : e + 1],
                b2s[0:1, e : e + 1],
            )
        nc.sync.dma_start(vout[:], pay[:])


def emit_phase_b(nc, tc, io):
    """One f32r M=4 stream (logits+s), vectorized gating, row log_softmax."""
    xc, wgt, vin, out = io["xc"], io["wgt"], io["vin"], io["out"]
    rings = [nc.sync, nc.scalar]
    with (
        tc.tile_pool(name="main", bufs=1) as pool,
        tc.tile_pool(name="psum", bufs=1, space="PSUM") as psum,
    ):
        # v and Wg arrive as [16, 128] n-major rows (contiguous 512B per
        # partition) and get transposed on the idle PE into the [128, 16]
        # layout the stationary operand needs
        vrow = pool.tile([16, E * P], F32)
        for e in range(E):
            nc.sync.dma_start(
                vrow[:, e * P : (e + 1) * P],
                vin[0:1, e * D : (e + 1) * D].rearrange("x (n p) -> n (x p)", p=P),
            )
        wgr = pool.tile([16, E * P], F32)
        for e in range(E):
            nc.scalar.dma_start(wgr[:, e * P : (e + 1) * P], wgt[e])
        csum = pool.tile([1, E], F32)
        nc.gpsimd.dma_start(csum[:], vin[0:1, 2 * D : 2 * D + E])

        # x: host-prearranged [128, NB*TB] (d = n*128 + p), 8KB/partition
        # contiguous per chunk; 4 chunks alternate the HWDGE rings so the
        # accumulating matmul stream pipelines behind the DMA
        x_sb = pool.tile([P, NB * TB], F32R)
        XCH = NB * TB // 4
        for k in range(4):
            rings[k % 2].dma_start(
                x_sb[:, k * XCH : (k + 1) * XCH], xc[:, k * XCH : (k + 1) * XCH]
            )

        # preload ACT tables (Exp, Ln) off the critical path
        warm = pool.tile([1, 2], F32)
        nc.gpsimd.memset(warm[:], 1.0)
        wz = pool.tile([1, 2], F32)
        nc.scalar.activation(wz[:], warm[:], AF.Exp)
        nc.scalar.activation(wz[:], warm[:], AF.Ln)

        ident = pool.tile([P, P], F32)
        make_identity(nc, ident[:])
        # m4[p, n, :] = [wg0 wg1 v0 v1] for d-block n (d = n*128 + p): one
        # M=4 f32r stream computes logits AND s together at 1 cycle/row
        m4 = pool.tile([P, NB, 4], F32R)
        for e in range(E):
            wtp = psum.tile([P, NB], F32, name=f"wtp_{e}", tag="tp16", bufs=2)
            nc.tensor.transpose(wtp[:], wgr[:, e * P : (e + 1) * P], ident[0:16, 0:16])
            nc.vector.tensor_copy(m4[:, :, e : e + 1], wtp[:, :, None])
            vtp = psum.tile([P, NB], F32, name=f"vtp_{e}", tag="tp16", bufs=2)
            nc.tensor.transpose(vtp[:], vrow[:, e * P : (e + 1) * P], ident[0:16, 0:16])
            nc.vector.tensor_copy(m4[:, :, 2 + e : 3 + e], vtp[:, :, None])
        # cb16[p, g, :] = [0, 0, c0, c1] for every group; plus const tiles
        cb16 = pool.tile([P, NG, 4], F32)
        nc.gpsimd.memset(cb16[:, :, 0:2], 0.0)
        for g in range(NG):
            nc.gpsimd.partition_broadcast(cb16[:, g, 2:4], csum[0:1, :])
        zz = pool.tile([P, NG], F32)
        nc.gpsimd.memset(zz[:], 0.0)
        one1 = pool.tile([P, 1], F32)
        nc.gpsimd.memset(one1[:], 1.0)

        ps4 = psum.tile([4, TB], F32)
        for n in range(NB):
            nc.tensor.matmul(
                ps4[:],
                m4[:, n, :],
                x_sb[:, n * TB : (n + 1) * TB],
                start=(n == 0),
                stop=(n == NB - 1),
            )
        sbl = pool.tile([4, TB], F32)
        nc.vector.tensor_copy(sbl[:], ps4[:])

        # gating, vectorized across all 4 token groups: t16[:, g, :] holds
        # [l0 l1 s0 s1] for tokens g*128..g*128+127
        t16_ps = psum.tile([P, NG, 4], F32)
        for g in range(NG):
            nc.tensor.transpose(
                t16_ps[:, g, :], sbl[0:4, g * P : (g + 1) * P], ident[0:4, 0:4]
            )
        t16 = pool.tile([P, NG, 4], F32)
        nc.vector.tensor_add(t16[:], t16_ps[:], cb16[:])
        dl = pool.tile([P, NG], F32)
        nc.vector.tensor_sub(dl[:, :, None], t16[:, :, 0:1], t16[:, :, 1:2])
        ndl = pool.tile([P, NG], F32)
        nc.vector.tensor_sub(ndl[:], zz[:], dl[:])
        mneg = pool.tile([P, NG], F32)
        nc.vector.tensor_tensor(mneg[:], dl[:], ndl[:], op=ALU.min)  # -|dl|
        eneg = pool.tile([P, NG], F32)
        nc.scalar.activation(eneg[:], mneg[:], AF.Exp)
        # prefetch the Ln table now; overlaps the DVE gating chain below
        nc.scalar.activation(wz[:], warm[:], AF.Ln)
        den = pool.tile([P, NG], F32)
        nc.vector.tensor_scalar_add(den[:], eneg[:], one1[:])
        gate = pool.tile([P, NG], F32)
        nc.vector.reciprocal(gate[:], den[:])  # = max softmax prob
        mask = pool.tile([P, NG], F32)
        nc.vector.tensor_tensor(mask[:], dl[:], zz[:], op=ALU.is_ge)
        sdiff = pool.tile([P, NG], F32)
        nc.vector.tensor_sub(sdiff[:, :, None], t16[:, :, 2:3], t16[:, :, 3:4])
        msd = pool.tile([P, NG], F32)
        nc.vector.tensor_mul(msd[:], mask[:], sdiff[:])
        ssel = pool.tile([P, NG], F32)
        nc.vector.tensor_add(ssel[:, :, None], msd[:, :, None], t16[:, :, 3:4])
        moe_sb = pool.tile([P, NG], F32)
        nc.vector.tensor_mul(moe_sb[:], gate[:], ssel[:])

        # row log_softmax over all 512 tokens, via PE transposes
        tp4 = psum.tile([NG, P], F32)
        nc.tensor.transpose(tp4[:], moe_sb[:], ident[:])
        sb4t = pool.tile([NG, P], F32)
        nc.vector.tensor_copy(sb4t[:], tp4[:])
        m4p = pool.tile([NG, 1], F32)
        nc.vector.reduce_max(m4p[:], sb4t[:], axis=AX.X)
        m1p = psum.tile([1, NG], F32, name="m1p", tag="t1", bufs=2)
        nc.tensor.transpose(m1p[:], m4p[:], ident[0:NG, 0:NG])
        negm2 = pool.tile([1, 1], F32)
        nc.vector.reduce_max(negm2[:], m1p[:], axis=AX.X, negate=True)
        negm4 = pool.tile([NG, 1], F32)
        nc.gpsimd.partition_broadcast(negm4[:], negm2[:])
        e4 = pool.tile([NG, P], F32)
        s4 = pool.tile([NG, 1], F32)
        nc.scalar.activation(e4[:], sb4t[:], AF.Exp, bias=negm4[:], accum_out=s4[:])
        # re-warm the Ln table (in case the Exp above evicted it); overlaps
        # the transpose+reduce on other engines
        nc.scalar.activation(wz[:], warm[:], AF.Ln)
        s1p = psum.tile([1, NG], F32, name="s1p", tag="t1", bufs=2)
        nc.tensor.transpose(s1p[:], s4[:], ident[0:NG, 0:NG])
        ssum = pool.tile([1, 1], F32)
        nc.vector.reduce_sum(ssum[:], s1p[:], axis=AX.X)
        logs = pool.tile([1, 1], F32)
        nc.scalar.activation(logs[:], ssum[:], AF.Ln)
        shift = pool.tile([1, 1], F32)
        nc.vector.tensor_sub(shift[:], negm2[:], logs[:])
        shift4 = pool.tile([NG, 1], F32)
        nc.gpsimd.partition_broadcast(shift4[:], shift[:])
        res4 = pool.tile([NG, P], F32)
        nc.vector.tensor_scalar_add(res4[:], sb4t[:], shift4[:])
        nc.sync.dma_start(out.rearrange("x (g p) -> g (x p)", p=P), res4[:])


_CACHED = {}


def build_program(which):
    if which in _CACHED:
        return _CACHED[which]
    nc = bacc.Bacc(
        "TRN2",
        target_bir_lowering=False,
        debug=False,
        enable_asserts=False,
        num_devices=NCORES,
    )
    if which == "a":
        io = {
            "w1c": nc.dram_tensor("w1c", [P, E * D], F32R, kind="ExternalInput").ap(),
            "w2c": nc.dram_tensor("w2c", [P, E * D], F32, kind="ExternalInput").ap(),
            "b1c": nc.dram_tensor("b1c", [1, E * HC], F32, kind="ExternalInput").ap(),
            "b2c": nc.dram_tensor("b2c", [1, E * DC], F32, kind="ExternalInput").ap(),
            "vout": nc.dram_tensor("vout", [1, VPART], F32, kind="ExternalOutput").ap(),
        }
        emit = emit_phase_a
    else:
        io = {
            "xc": nc.dram_tensor("xc", [P, NB * TB], F32R, kind="ExternalInput").ap(),
            "wgt": nc.dram_tensor("wgt", [E, NB, P], F32, kind="ExternalInput").ap(),
            "vin": nc.dram_tensor("vin", [1, VPART], F32, kind="ExternalInput").ap(),
            "out": nc.dram_tensor("out", [1, TB], F32, kind="ExternalOutput").ap(),
        }
        emit = emit_phase_b
    with tile.TileContext(nc) as tc:
        emit(nc, tc, io)
    nc.compile()
    _CACHED[which] = nc
    return nc


def shard_inputs_a(Wg, W1, b1, W2, b2):
    W1 = np.asarray(W1, np.float32)
    b1 = np.asarray(b1, np.float32)
    W2 = np.asarray(W2, np.float32)
    b2 = np.asarray(b2, np.float32)
    in_maps = []
    for c in range(NCORES):
        hs, he = c * HC, (c + 1) * HC
        # w1c[p, e*D+d] = W1[e, d, hs+p];  w2c[p, e*D+d] = W2[e, hs+p, d]
        in_maps.append(
            {
                "w1c": np.ascontiguousarray(
                    W1[:, :, hs:he].transpose(2, 0, 1).reshape(HC, E * D)
                ),
                "w2c": np.ascontiguousarray(
                    W2[:, hs:he, :].transpose(1, 0, 2).reshape(HC, E * D)
                ),
                "b1c": np.ascontiguousarray(b1[:, hs:he].reshape(1, E * HC)),
                "b2c": np.ascontiguousarray(
                    b2[:, c * DC : (c + 1) * DC].reshape(1, E * DC)
                ),
            }
        )
    return in_maps


def shard_inputs_b(x, Wg, vpart_sum):
    x = np.asarray(x, np.float32)
    Wg = np.asarray(Wg, np.float32)
    # wgt[e, n, p] = Wg[n*128+p, e]  (d = n*128 + p decomposition)
    wgt = np.ascontiguousarray(Wg.reshape(NB, P, E).transpose(2, 0, 1))
    in_maps = []
    for c in range(NCORES):
        row = c % B
        # xc[p, n*TB + t] = x[row, t, n*128 + p]
        xr = np.ascontiguousarray(
            x[row].reshape(TB, NB, P).transpose(2, 1, 0).reshape(P, NB * TB)
        )
        in_maps.append({"xc": xr, "wgt": wgt, "vin": vpart_sum})
    return in_maps


def run_a(in_maps, **kwargs):
    return bass_utils.run_bass_kernel_spmd(
        build_program("a"), in_maps, core_ids=list(range(NCORES)), **kwargs
    )


def run_b(in_maps, **kwargs):
    return bass_utils.run_bass_kernel_spmd(
        build_program("b"), in_maps, core_ids=list(range(NCORES)), **kwargs
    )


def kernel(x, Wg, W1, b1, W2, b2):
    res_a = run_a(shard_inputs_a(Wg, W1, b1, W2, b2))
    # cross-core combine: sum of the 8 per-core partials (the gather/reshard
    # step between the two launches; 16KB, no model math beyond the reduction)
    vpart = np.sum([res_a.results[c]["vout"] for c in range(NCORES)], axis=0)
    vpart = np.ascontiguousarray(vpart, np.float32)
    res_b = run_b(shard_inputs_b(x, Wg, vpart))
    return np.concatenate([res_b.results[b]["out"] for b in range(B)], axis=0)


# revision 7
# speedup vs baseline: 1.1605x; 1.0098x over previous
"""Trainium2 Bass kernel for nn_ExampleModel_1116691497724 (moe_routing).

Math: the reference returns log_softmax_T( sum_D(moe_out) ), and sum_D
collapses the expert FFN to a dot product:
    sum_d (h @ W2[e] + b2[e]) = h . w2sum[e] + sum(b2[e]),  w2sum[e] = W2[e] @ 1
    (x @ W1[e] + b1[e]) . w2sum[e] = x . v[e] + c[e]
with v[e] = W1[e] @ w2sum[e]  (a [D] vector) and scalar
c[e] = b1[e].w2sum[e] + sum(b2[e]).  Then per token:
    s_e = x . v[e] + c[e],  logits = x @ Wg
    moe_sum = max(softmax(logits)) * s_argmax(logits)
    out = log_softmax over tokens (per batch row) of moe_sum.

Distribution over 8 cores, two launches (a single ncfw collective costs far
more in barrier/trigger latency on this runtime than a second launch, so the
16KB cross-core combine happens on the host between launches; the host does
only that partial sum, all real math stays on device):
  launch A (expert-parallel over H): core c reduces W2[:, 128c:128c+128, :]
    and computes partial v from the matching W1 columns -> outputs
    [v0 | v1 | c0 c1] partials (16KB); host sums.
  launch B (token-parallel): core c owns batch row c%4 (512 tokens); one
    f32r M=4 stream computes logits AND s per token (1 cycle/row vs fp32's
    4); gate/select vectorized across all 4 token groups via
    gate = 1/(1+exp(-|l0-l1|)); row log_softmax via PE transposes.

All big HBM tensors are laid out host-side to exactly match SBUF layout
([128, free] with long contiguous per-partition runs) so each dma_start
emits few large descriptors and streams near HBM line rate.
"""

import sys

import numpy as np

for _p in ("/opt/trn_rl_repo",):
    if _p not in sys.path:
        sys.path.append(_p)

import concourse.bass as bass  # noqa: E402
import concourse.mybir as mybir  # noqa: E402
import concourse.tile as tile  # noqa: E402
from concourse import bacc, bass_utils  # noqa: E402
from concourse.masks import make_identity  # noqa: E402

# Problem shape (hardcoded per spec).
B, T, D, H, E = 4, 512, 2048, 1024, 2
P = 128
NCORES = 8
TB = T  # tokens per core = one batch row
NB = D // P  # 16 d-blocks
HC = H // NCORES  # 128 h-chunk per expert per core
NG = TB // P  # 4 token groups per core
DC = D // NCORES  # 256 b2 columns per core
VK = 4  # v computed in VK chunks of D/VK columns
F32 = mybir.dt.float32
F32R = mybir.dt.float32r
AX = mybir.AxisListType
AF = mybir.ActivationFunctionType
ALU = mybir.AluOpType

VPART = 2 * D + 2  # launch A output: v0 | v1 | c0 c1


def emit_phase_a(nc, tc, io):
    """w2sum + partial v for this core's H-chunk -> vpart [1, 2D+2]."""
    w1c, w2c, b1c, b2c, vout = io["w1c"], io["w2c"], io["b1c"], io["b2c"], io["vout"]
    rings = [nc.sync, nc.scalar]
    with (
        tc.tile_pool(name="main", bufs=1) as pool,
        tc.tile_pool(name="psum", bufs=1, space="PSUM") as psum,
    ):
        # tiny bias rows on the SWDGE ring so they never head-of-line block
        # the big HWDGE streams
        b1_sb = pool.tile([1, E * HC], F32)
        nc.gpsimd.dma_start(b1_sb[:], b1c)
        b2_sb = pool.tile([1, E * DC], F32)
        nc.gpsimd.dma_start(b2_sb[:], b2c)

        # W2 first (gates the reduce -> w2sum -> everything), then W1.
        # Both are host-prearranged [128, E*D]; ring r carries expert r's
        # halves as single 1MB transfers (8KB/partition contiguous runs ->
        # few large descriptors, near line rate).
        w2_sb = pool.tile([P, E * D], F32)
        w1r = pool.tile([P, E * D], F32R)
        for e in range(E):
            rings[e].dma_start(w2_sb[:, e * D : (e + 1) * D], w2c[:, e * D : (e + 1) * D])
        for e in range(E):
            rings[e].dma_start(w1r[:, e * D : (e + 1) * D], w1c[:, e * D : (e + 1) * D])

        # b1 row -> partition-major [128, E] via PE transpose (identity [1,1])
        one1 = pool.tile([1, 1], F32)
        nc.gpsimd.memset(one1[:], 1.0)
        b1t_ps = psum.tile([P, E], F32)
        for e in range(E):
            nc.tensor.transpose(
                b1t_ps[:, e : e + 1], b1_sb[0:1, e * HC : (e + 1) * HC], one1[:]
            )
        b1p = pool.tile([P, E], F32)
        nc.vector.tensor_copy(b1p[:], b1t_ps[:])

        # per-expert reduce as each W2 half lands (overlaps the W1 DMA)
        w2s = pool.tile([P, E], F32)
        w2s_r = pool.tile([P, E], F32R)
        for e in range(E):
            nc.vector.reduce_sum(
                w2s[:, e : e + 1], w2_sb[:, e * D : (e + 1) * D], axis=AX.X
            )
            nc.vector.tensor_copy(w2s_r[:, e : e + 1], w2s[:, e : e + 1])
        b2s = pool.tile([1, E], F32)
        for e in range(E):
            nc.vector.reduce_sum(
                b2s[0:1, e : e + 1], b2_sb[0:1, e * DC : (e + 1) * DC], axis=AX.X
            )

        pay = pool.tile([1, VPART], F32)
        b1dot = psum.tile([1, E], F32)
        DK = D // VK
        for e in range(E):
            for k in range(VK):
                vch = psum.tile([1, DK], F32, name="vch", tag="vch", bufs=2)
                nc.tensor.matmul(
                    vch[:],
                    w2s_r[:, e : e + 1],
                    w1r[:, e * D + k * DK : e * D + (k + 1) * DK],
                    start=True,
                    stop=True,
                )
                dst = pay[0:1, e * D + k * DK : e * D + (k + 1) * DK]
                if k % 2 == 0:
                    nc.vector.tensor_copy(dst, vch[:])
                else:
                    nc.scalar.copy(dst, vch[:])
            nc.tensor.matmul(
                b1dot[0:1, e : e + 1],
                w2s[:, e : e + 1],
                b1p[:, e : e + 1],
                start=True,
                stop=True,
            )
            nc.vector.tensor_add(
                pay[0:1, 2 * D + e : 2 * D + e + 1],
                b1dot[0:1, e : e + 1],
                b2s[0:1, e : e + 1],
            )
        nc.sync.dma_start(vout[:], pay[:])


def emit_phase_b(nc, tc, io):
    """One f32r M=4 stream (logits+s), vectorized gating, row log_softmax."""
    xc, wgt, vin, out = io["xc"], io["wgt"], io["vin"], io["out"]
    rings = [nc.sync, nc.scalar]
    with (
        tc.tile_pool(name="main", bufs=1) as pool,
        tc.tile_pool(name="psum", bufs=1, space="PSUM") as psum,
    ):
        # v and Wg arrive as [16, 128] n-major rows (contiguous 512B per
        # partition) and get transposed on the idle PE into the [128, 16]
        # layout the stationary operand needs
        vrow = pool.tile([16, E * P], F32)
        for e in range(E):
            nc.sync.dma_start(
                vrow[:, e * P : (e + 1) * P],
                vin[0:1, e * D : (e + 1) * D].rearrange("x (n p) -> n (x p)", p=P),
            )
        wgr = pool.tile([16, E * P], F32)
        for e in range(E):
            nc.scalar.dma_start(wgr[:, e * P : (e + 1) * P], wgt[e])
        csum = pool.tile([1, E], F32)
        nc.gpsimd.dma_start(csum[:], vin[0:1, 2 * D : 2 * D + E])

        # x: host-prearranged [128, NB*TB] (d = n*128 + p), 4KB/partition
        # contiguous per chunk; 8 chunks of 2 d-blocks alternate the HWDGE
        # rings so the ordered accumulating matmul stream pipelines behind
        # the DMA and only a small tail runs after the last chunk lands
        x_sb = pool.tile([P, NB * TB], F32R)
        XCH = NB * TB // 8
        for k in range(8):
            rings[k % 2].dma_start(
                x_sb[:, k * XCH : (k + 1) * XCH], xc[:, k * XCH : (k + 1) * XCH]
            )

        # preload the Exp ACT table off the critical path (Ln is loaded late,
        # right after the last Exp use, to avoid ping-pong reloads)
        warm = pool.tile([1, 2], F32)
        nc.gpsimd.memset(warm[:], 1.0)
        wz = pool.tile([1, 2], F32)
        nc.scalar.activation(wz[:], warm[:], AF.Exp)

        ident = pool.tile([P, P], F32)
        make_identity(nc, ident[:])
        # m4[p, n, :] = [wg0 wg1 v0 v1] for d-block n (d = n*128 + p): one
        # M=4 f32r stream computes logits AND s together at 1 cycle/row
        m4 = pool.tile([P, NB, 4], F32R)
        for e in range(E):
            wtp = psum.tile([P, NB], F32, name=f"wtp_{e}", tag="tp16", bufs=2)
            nc.tensor.transpose(wtp[:], wgr[:, e * P : (e + 1) * P], ident[0:16, 0:16])
            nc.vector.tensor_copy(m4[:, :, e : e + 1], wtp[:, :, None])
            vtp = psum.tile([P, NB], F32, name=f"vtp_{e}", tag="tp16", bufs=2)
            nc.tensor.transpose(vtp[:], vrow[:, e * P : (e + 1) * P], ident[0:16, 0:16])
            nc.vector.tensor_copy(m4[:, :, 2 + e : 3 + e], vtp[:, :, None])
        # cb16[p, g, :] = [0, 0, c0, c1] for every group; plus const tiles
        cb16 = pool.tile([P, NG, 4], F32)
        nc.gpsimd.memset(cb16[:, :, 0:2], 0.0)
        for g in range(NG):
            nc.gpsimd.partition_broadcast(cb16[:, g, 2:4], csum[0:1, :])
        zz = pool.tile([P, NG], F32)
        nc.gpsimd.memset(zz[:], 0.0)
        one1 = pool.tile([P, 1], F32)
        nc.gpsimd.memset(one1[:], 1.0)

        ps4 = psum.tile([4, TB], F32)
        for n in range(NB):
            nc.tensor.matmul(
                ps4[:],
                m4[:, n, :],
                x_sb[:, n * TB : (n + 1) * TB],
                start=(n == 0),
                stop=(n == NB - 1),
            )
        sbl = pool.tile([4, TB], F32)
        nc.vector.tensor_copy(sbl[:], ps4[:])

        # gating, vectorized across all 4 token groups: t16[:, g, :] holds
        # [l0 l1 s0 s1] for tokens g*128..g*128+127
        t16_ps = psum.tile([P, NG, 4], F32)
        for g in range(NG):
            nc.tensor.transpose(
                t16_ps[:, g, :], sbl[0:4, g * P : (g + 1) * P], ident[0:4, 0:4]
            )
        t16 = pool.tile([P, NG, 4], F32)
        nc.vector.tensor_add(t16[:], t16_ps[:], cb16[:])
        dl = pool.tile([P, NG], F32)
        nc.vector.tensor_sub(dl[:, :, None], t16[:, :, 0:1], t16[:, :, 1:2])
        ndl = pool.tile([P, NG], F32)
        nc.vector.tensor_sub(ndl[:], zz[:], dl[:])
        mneg = pool.tile([P, NG], F32)
        nc.vector.tensor_tensor(mneg[:], dl[:], ndl[:], op=ALU.min)  # -|dl|
        eneg = pool.tile([P, NG], F32)
        nc.scalar.activation(eneg[:], mneg[:], AF.Exp)
        den = pool.tile([P, NG], F32)
        nc.vector.tensor_scalar_add(den[:], eneg[:], one1[:])
        gate = pool.tile([P, NG], F32)
        nc.vector.reciprocal(gate[:], den[:])  # = max softmax prob
        mask = pool.tile([P, NG], F32)
        nc.vector.tensor_tensor(mask[:], dl[:], zz[:], op=ALU.is_ge)
        sdiff = pool.tile([P, NG], F32)
        nc.vector.tensor_sub(sdiff[:, :, None], t16[:, :, 2:3], t16[:, :, 3:4])
        msd = pool.tile([P, NG], F32)
        nc.vector.tensor_mul(msd[:], mask[:], sdiff[:])
        ssel = pool.tile([P, NG], F32)
        nc.vector.tensor_add(ssel[:, :, None], msd[:, :, None], t16[:, :, 3:4])
        moe_sb = pool.tile([P, NG], F32)
        nc.vector.tensor_mul(moe_sb[:], gate[:], ssel[:])

        # row log_softmax over all 512 tokens, via PE transposes
        tp4 = psum.tile([NG, P], F32)
        nc.tensor.transpose(tp4[:], moe_sb[:], ident[:])
        sb4t = pool.tile([NG, P], F32)
        nc.vector.tensor_copy(sb4t[:], tp4[:])
        m4p = pool.tile([NG, 1], F32)
        nc.vector.reduce_max(m4p[:], sb4t[:], axis=AX.X)
        m1p = psum.tile([1, NG], F32, name="m1p", tag="t1", bufs=2)
        nc.tensor.transpose(m1p[:], m4p[:], ident[0:NG, 0:NG])
        negm2 = pool.tile([1, 1], F32)
        nc.vector.reduce_max(negm2[:], m1p[:], axis=AX.X, negate=True)
        negm4 = pool.tile([NG, 1], F32)
        nc.gpsimd.partition_broadcast(negm4[:], negm2[:])
        e4 = pool.tile([NG, P], F32)
        s4 = pool.tile([NG, 1], F32)
        nc.scalar.activation(e4[:], sb4t[:], AF.Exp, bias=negm4[:], accum_out=s4[:])
        # load the Ln table now (first Ln use); overlaps the transpose+reduce
        nc.scalar.activation(wz[:], warm[:], AF.Ln)
        s1p = psum.tile([1, NG], F32, name="s1p", tag="t1", bufs=2)
        nc.tensor.transpose(s1p[:], s4[:], ident[0:NG, 0:NG])
        ssum = pool.tile([1, 1], F32)
        nc.vector.reduce_sum(ssum[:], s1p[:], axis=AX.X)
        logs = pool.tile([1, 1], F32)
        nc.scalar.activation(logs[:], ssum[:], AF.Ln)
        shift = pool.tile([1, 1], F32)
        nc.vector.tensor_sub(shift[:], negm2[:], logs[:])
        shift4 = pool.tile([NG, 1], F32)
        nc.gpsimd.partition_broadcast(shift4[:], shift[:])
        res4 = pool.tile([NG, P], F32)
        nc.vector.tensor_scalar_add(res4[:], sb4t[:], shift4[:])
        nc.sync.dma_start(out.rearrange("x (g p) -> g (x p)", p=P), res4[:])


_CACHED = {}


def build_program(which):
    if which in _CACHED:
        return _CACHED[which]
    nc = bacc.Bacc(
        "TRN2",
        target_bir_lowering=False,
        debug=False,
        enable_asserts=False,
        num_devices=NCORES,
    )
    if which == "a":
        io = {
            "w1c": nc.dram_tensor("w1c", [P, E * D], F32R, kind="ExternalInput").ap(),
            "w2c": nc.dram_tensor("w2c", [P, E * D], F32, kind="ExternalInput").ap(),
            "b1c": nc.dram_tensor("b1c", [1, E * HC], F32, kind="ExternalInput").ap(),
            "b2c": nc.dram_tensor("b2c", [1, E * DC], F32, kind="ExternalInput").ap(),
            "vout": nc.dram_tensor("vout", [1, VPART], F32, kind="ExternalOutput").ap(),
        }
        emit = emit_phase_a
    else:
        io = {
            "xc": nc.dram_tensor("xc", [P, NB * TB], F32R, kind="ExternalInput").ap(),
            "wgt": nc.dram_tensor("wgt", [E, NB, P], F32, kind="ExternalInput").ap(),
            "vin": nc.dram_tensor("vin", [1, VPART], F32, kind="ExternalInput").ap(),
            "out": nc.dram_tensor("out", [1, TB], F32, kind="ExternalOutput").ap(),
        }
        emit = emit_phase_b
    with tile.TileContext(nc) as tc:
        emit(nc, tc, io)
    nc.compile()
    _CACHED[which] = nc
    return nc


def shard_inputs_a(Wg, W1, b1, W2, b2):
    W1 = np.asarray(W1, np.float32)
    b1 = np.asarray(b1, np.float32)
    W2 = np.asarray(W2, np.float32)
    b2 = np.asarray(b2, np.float32)
    in_maps = []
    for c in range(NCORES):
        hs, he = c * HC, (c + 1) * HC
        # w1c[p, e*D+d] = W1[e, d, hs+p];  w2c[p, e*D+d] = W2[e, hs+p, d]
        in_maps.append(
            {
                "w1c": np.ascontiguousarray(
                    W1[:, :, hs:he].transpose(2, 0, 1).reshape(HC, E * D)
                ),
                "w2c": np.ascontiguousarray(
                    W2[:, hs:he, :].transpose(1, 0, 2).reshape(HC, E * D)
                ),
                "b1c": np.ascontiguousarray(b1[:, hs:he].reshape(1, E * HC)),
                "b2c": np.ascontiguousarray(
                    b2[:, c * DC : (c + 1) * DC].reshape(1, E * DC)
                ),
            }
        )
    return in_maps


def shard_inputs_b(x, Wg, vpart_sum):
    x = np.asarray(x, np.float32)
    Wg = np.asarray(Wg, np.float32)
    # wgt[e, n, p] = Wg[n*128+p, e]  (d = n*128 + p decomposition)
    wgt = np.ascontiguousarray(Wg.reshape(NB, P, E).transpose(2, 0, 1))
    in_maps = []
    for c in range(NCORES):
        row = c % B
        # xc[p, n*TB + t] = x[row, t, n*128 + p]
        xr = np.ascontiguousarray(
            x[row].reshape(TB, NB, P).transpose(2, 1, 0).reshape(P, NB * TB)
        )
        in_maps.append({"xc": xr, "wgt": wgt, "vin": vpart_sum})
    return in_maps


def run_a(in_maps, **kwargs):
    return bass_utils.run_bass_kernel_spmd(
        build_program("a"), in_maps, core_ids=list(range(NCORES)), **kwargs
    )


def run_b(in_maps, **kwargs):
    return bass_utils.run_bass_kernel_spmd(
        build_program("b"), in_maps, core_ids=list(range(NCORES)), **kwargs
    )


def kernel(x, Wg, W1, b1, W2, b2):
    res_a = run_a(shard_inputs_a(Wg, W1, b1, W2, b2))
    # cross-core combine: sum of the 8 per-core partials (the gather/reshard
    # step between the two launches; 16KB, no model math beyond the reduction)
    vpart = np.sum([res_a.results[c]["vout"] for c in range(NCORES)], axis=0)
    vpart = np.ascontiguousarray(vpart, np.float32)
    res_b = run_b(shard_inputs_b(x, Wg, vpart))
    return np.concatenate([res_b.results[b]["out"] for b in range(B)], axis=0)
